# revision 61
# baseline (speedup 1.0000x reference)
"""Trainium2 Bass kernel for gated multi-head attention with pair bias.

Reference computation (B=2, S=2048, C_IN=512, H=8, C=64):
    q,k,v = heads(x @ Wq), heads(x @ Wk), heads(x @ Wv)
    logits = q k^T / sqrt(C) + bias + mask_offset
    attn   = softmax(logits)
    o      = attn @ v
    out    = (sigmoid(x @ Wg + bg) * concat(o)) @ Wo + bo

Sharding: 8 cores = 2 batches x 4 head-pairs. Core c handles batch c//4,
heads (2*(c%4), 2*(c%4)+1). Each core computes a partial output (sum over
its two heads); the host sums 4 partials per batch and adds bo.

Structure:
  - The key mask keeps only ~half the 2048 keys. The host compacts keys:
    x rows and exp(bias^T) rows are gathered to the kept keys and padded
    to KT tiles of 128 (padding rows have exp-bias 0 so they contribute
    nothing to numerator or denominator). All per-key work runs on KT
    tiles. All math stays bf16/fp32 — fp8 anywhere in the attention value
    path costs ~2-4e-2 relative error (the pair bias peaks the softmax,
    so few effective keys average the quantization noise).
  - softmax skips max-subtraction (logits are O(+-8); fp32 exp is safe)
    and uses exp(qk) * exp(bias) with exp(bias^T) precomputed on the host
    in bf16; the denominator comes from a ones column appended to v.
  - Per key tile, BOTH heads' qk run as two row-group matmuls (K=64 in
    rows 0:64 / 64:128) into one [128,1024] PSUM tile, so a single ACT
    exp covers both heads.
  - PE issue order is software-pipelined: attention-V matmuls trail the
    qk of the NEXT key tile, and v/q/gate projection chunks are trickled
    into the attention stream, so the PE never idles waiting on
    exp->mult. Sustained PE duty keeps the HAM clock gate at full speed
    (dropping duty re-throttles the array to half clock).
  - The softmax rowsum is broadcast across partitions on GPSIMD and
    folded into the gate BEFORE the output projection, so both heads
    accumulate into one PSUM tile and the fp32 per-tile combine (and the
    rowsum DRAM round trip) disappear.
  - Output projection is interleaved into the attention stream one
    q-chunk behind.
"""

import math
import sys
import threading

import numpy as np

sys.path.insert(0, "/opt/trn_rl_repo")

import ml_dtypes

import concourse.bass as bass
import concourse.tile as tile
from concourse import mybir
from concourse.bass_utils import run_bass_kernel_spmd
from concourse.alu_op_type import AluOpType

# ---------------------------------------------------------------------------
# This toolchain's walrus encodes at most ONE semaphore wait per Drain/CTRL
# instruction; Tile's end-of-kernel drain can carry several (one per DMA
# queue). Split them across a chain of single-wait drains.
# ---------------------------------------------------------------------------


_NOP_UID = [0]


def _split_multi_waits(nc):
    """Rewrite every instruction carrying >1 sem waits: keep one wait on the
    instruction, hoist the others onto same-engine NoOps inserted right
    before it (engine streams execute in order, so this is equivalent)."""
    for fn in nc.m.functions:
        for bb in fn.blocks:
            insts = list(bb.instructions)
            out = []
            changed = False
            for inst in insts:
                si = inst.sync_info
                if si is not None and len(si.on_wait) > 1:
                    changed = True
                    waits = list(si.on_wait)
                    si.on_wait = waits[:1]
                    for w in waits[1:]:
                        _NOP_UID[0] += 1
                        nop = mybir.InstNoOp(
                            name=f"waitsplit-{_NOP_UID[0]}",
                            engine=inst.engine,
                            ins=[],
                            outs=[],
                        )
                        nop.sync_info = mybir.SyncInfo(on_wait=[w], on_update=[])
                        out.append(nop)
                out.append(inst)
            if changed:
                bb.instructions = out


def _drain_and_barrier_split(self, tick_clock, wait_clock):
    from concourse.vector_clock import ScopedClock

    drain_inst = self.nc.sync.drain()
    wait_clock.add_sem_waits(
        drain_inst.ins, ScopedClock({None: tick_clock.global_clock})
    )
    si = drain_inst.ins.sync_info
    if si is not None and len(si.on_wait) > 1:
        extra = list(si.on_wait[1:])
        si.on_wait = list(si.on_wait[:1])
        for w in extra:
            d2 = self.nc.sync.drain()
            d2.ins.sync_info = mybir.SyncInfo(on_wait=[w], on_update=[])

    self.nc.all_engine_barrier()
    assert self.sems is not None
    popped = self.nc._tile_sem_poison_stack.pop()
    assert popped is self._sem_poison
    self.nc.clear_and_free_semaphores(list(self.sems.allocated().values()))
    self.nc.all_engine_barrier()

    _split_multi_waits(self.nc)


tile.TileContext._drain_and_barrier = _drain_and_barrier_split

BF16 = mybir.dt.bfloat16
F32 = mybir.dt.float32
NBF = ml_dtypes.bfloat16

B, S, C_IN, H, C = 2, 2048, 512, 8, 64
P = 128
NCI = C_IN // P  # 4 contraction chunks
QH = 512  # q-chunk per attention iteration
NJ = S // QH  # 4
NQT = S // P  # 16 q tiles
QTJ = QH // P  # 4 q tiles per j-chunk

Exp = mybir.ActivationFunctionType.Exp
Tanh = mybir.ActivationFunctionType.Tanh


def _build_nc(KT):
    """KT = number of 128-key tiles after mask compaction (even)."""
    KPP = KT * P
    nc = bass.Bass("TRN2")

    # host layouts are partition-major: single contiguous DMAs
    x_t = nc.dram_tensor("xt", [P, NCI, S], BF16, kind="ExternalInput")
    xk_t = nc.dram_tensor("xk", [P, NCI, KPP], BF16, kind="ExternalInput")
    w_t = nc.dram_tensor("wall", [P, 4, NCI, P], BF16, kind="ExternalInput")
    cst_t = nc.dram_tensor("csts", [P, 2], F32, kind="ExternalInput")
    eb_t = nc.dram_tensor("ebias", [KT, P, NJ, 2, QH], BF16, kind="ExternalInput")
    wo_t = nc.dram_tensor("wo", [P, C_IN], BF16, kind="ExternalInput")
    out_t = nc.dram_tensor("out", [S, C_IN], BF16, kind="ExternalOutput")
    # last q-chunk ships unnormalized per-head Wo partials + rowsums; the
    # host applies 1/rowsum there, cutting the round trip out of the tail
    out2_t = nc.dram_tensor("out2", [P, QTJ, 2, C_IN], BF16, kind="ExternalOutput")
    rs2_t = nc.dram_tensor("rsum2", [2, QH], F32, kind="ExternalOutput")

    from contextlib import ExitStack

    with tile.TileContext(nc) as tc, ExitStack() as ctx:
        const = ctx.enter_context(tc.tile_pool(name="const", bufs=1))

        # ---------------- persistent SBUF tiles ----------------
        # DMA order matters: the first q-proj matmul needs only wq + the
        # ci=0 slice of xt cols 0:512, so those land first.
        wall_sb = const.tile([P, 4, NCI, P], BF16, tag="wall")
        nc.sync.dma_start(wall_sb[:, 0:1, :, :], w_t[:, 0:1, :, :])
        xt_sb = const.tile([P, NCI, S], BF16, tag="xt")
        for ci in range(NCI):
            nc.sync.dma_start(xt_sb[:, ci : ci + 1, 0:QH],
                              x_t[:, ci : ci + 1, 0:QH])
        nc.sync.dma_start(wall_sb[:, 1:4, :, :], w_t[:, 1:4, :, :])
        xk_sb = const.tile([P, NCI, KPP], BF16, tag="xk")
        nc.sync.dma_start(xk_sb[:, :, 0 : KPP // 2], xk_t[:, :, 0 : KPP // 2])
        nc.sync.dma_start(xk_sb[:, :, KPP // 2 : KPP], xk_t[:, :, KPP // 2 : KPP])
        cst_sb = const.tile([P, 2], F32, tag="csts")
        nc.sync.dma_start(cst_sb[:], cst_t[:])
        wo_st = const.tile([P, C_IN], BF16, tag="wo")
        nc.sync.dma_start(wo_st[:], wo_t[:])
        # the bulk of x (cols 512:2048) is issued inside the j=0 loop so the
        # exp-bias stream gets DMA priority at the start
        bgv_sb = cst_sb[:, 0:1]
        one_sb = cst_sb[:, 1:2]

        qT = const.tile([P, S], BF16, tag="qT")
        kT = const.tile([P, KPP], BF16, tag="kT")
        gT = [const.tile([C, S], BF16, tag=f"gT{h}", name=f"gT{h}") for h in range(2)]
        vm = [const.tile([P, 2, 2 * (C + 1)], BF16, tag=f"vm{t}", name=f"vm{t}")
              for t in range(KT // 2)]
        # unnormalized gate*o (bf16, heads stacked on rows); 1/rowsum is
        # applied after the output projection (per-partition scalar there)
        goun2 = const.tile([P, S], BF16, tag="goun2")
        rrec = [const.tile([P, NQT], F32, tag=f"rrec{h}", name=f"rrec{h}")
                for h in range(2)]
        obj2 = const.tile([P, QTJ, 2, C_IN], BF16, tag="obj2")

        with (
            tc.tile_pool(name="spp", bufs=2, space="PSUM") as spp,
            tc.tile_pool(name="opp", bufs=2, space="PSUM") as opp,
            tc.tile_pool(name="ebp", bufs=6) as ebp,
            tc.tile_pool(name="ptp", bufs=3) as ptp,
            tc.tile_pool(name="pexp", bufs=3) as pexp,
            tc.tile_pool(name="epi", bufs=4) as epi,
            tc.tile_pool(name="drp", bufs=2, space="DRAM") as drp,
            tc.tile_pool(name="outs", bufs=2) as outs,
        ):
            # -------- projection emitters (called interleaved) --------
            def proj_chunk(wi, dest, sl, src=None):
                pp = spp.tile([P, 1024], F32, tag="sp")
                n = sl.stop - sl.start
                src = xt_sb if src is None else src
                for ci in range(NCI):
                    nc.tensor.matmul(
                        pp[:, 0:n], wall_sb[:, wi, ci, :], src[:, ci, sl],
                        start=(ci == 0), stop=(ci == NCI - 1),
                    )
                nc.vector.tensor_copy(dest[:, sl], pp[:, 0:n])

            def gate_chunk(ch):
                sl = slice(ch * QH, (ch + 1) * QH)
                pp = spp.tile([P, 1024], F32, tag="sp")
                for ci in range(NCI):
                    nc.tensor.matmul(
                        pp[:, 0:QH], wall_sb[:, 2, ci, :], xt_sb[:, ci, sl],
                        start=(ci == 0), stop=(ci == NCI - 1),
                    )
                # sigmoid(v) = 0.5 + 0.5*tanh(v/2); Tanh shares the ACT table
                # set with Exp so there is no mid-kernel table switch.
                for h in range(2):
                    hp = slice(C * h, C * (h + 1))
                    nc.scalar.activation(
                        gT[h][:, sl], pp[hp, 0:QH], Tanh,
                        bias=bgv_sb[hp, :], scale=0.5,
                    )
                    nc.vector.tensor_scalar(
                        gT[h][:, sl], gT[h][:, sl], 0.5, 0.5,
                        AluOpType.mult, AluOpType.add,
                    )

            def v_chunk(kt):
                ktsl = slice(kt * P, (kt + 1) * P)
                pv = spp.tile([P, 1024], F32, tag="sp")
                for ci in range(NCI):
                    nc.tensor.matmul(
                        pv[:, 0:P], xk_sb[:, ci, ktsl], wall_sb[:, 3, ci, :],
                        start=(ci == 0), stop=(ci == NCI - 1),
                    )
                v = vm[kt // 2]
                t = kt % 2
                for h in range(2):
                    o = (C + 1) * h
                    nc.vector.tensor_copy(v[:, t, o : o + C], pv[:, C * h : C * (h + 1)])
                    nc.vector.tensor_copy(v[:, t, o + C : o + C + 1], one_sb[:])

            # upfront: q chunk 0 and k (needed before attention j=0);
            # v / q chunks 1..3 / gate trickle into the attention stream.
            proj_chunk(0, qT, slice(0, QH))
            for ch in range(math.ceil(KPP / QH)):
                proj_chunk(1, kT, slice(ch * QH, min((ch + 1) * QH, KPP)), xk_sb)
            v_chunk(0)
            v_chunk(1)

            # ordering constraints: v_chunk(kt) before av(kt) [emitted kt+2];
            # gate_chunk(c) before epilogue(j=c); q chunk c before qk(j=c);
            # q/gate chunks past col 512 also need the bulk x DMA, so they
            # sit later in the queue. Queue is drained 2 items/iteration.
            vf = [lambda kt=kt: v_chunk(kt) for kt in range(2, KT)]
            qf = [lambda c=c: proj_chunk(0, qT, slice(c * QH, (c + 1) * QH))
                  for c in range(1, NJ)]
            gf = [lambda c=c: gate_chunk(c) for c in range(NJ)]
            # j0 drains the FIFO (v chunks have per-iteration deadlines);
            # later chunks get real PE work at scheduled slots so density
            # holds where the FIFO would have run dry (gate_chunk(c) must
            # precede epilogue(j=c), q chunk c must precede qk(j=c)).
            # q chunk 1 sits two slots before the j0/j1 boundary so its
            # projection completes with margin before qk(j1) consumes it
            if vf:
                fillers = vf[:-1] + [qf[0], vf[-1], gf[0]]
            else:
                fillers = [qf[0], gf[0]]
            sched = {(1, 1): gf[1], (1, 5): qf[1], (2, 1): gf[2],
                     (2, 5): qf[2], (3, 1): gf[3]}

            Copy = mybir.ActivationFunctionType.Copy

            def out_qtile(jj, t, obj):
                # per-head Wo partials; the per-head 1/rowsum scale applies
                # after the projection (q sits on partitions there). The
                # po0*r0 goes through ACT (Copy with per-partition scale) to
                # keep DVE off the critical path.
                qq = jj * QTJ + t
                qsl = slice(qq * P, (qq + 1) * P)
                po = spp.tile([P, 1024], F32, tag="sp")
                nc.tensor.matmul(po[:, 0:512], goun2[0:C, qsl], wo_st[0:C, :],
                                 start=True, stop=True)
                nc.tensor.matmul(po[:, 512:1024], goun2[C:P, qsl], wo_st[C:P, :],
                                 start=True, stop=True)
                if jj == NJ - 1:
                    # last chunk: raw per-head partials, host normalizes;
                    # per-tile DMA so the drain overlaps
                    nc.scalar.copy(obj2[:, t, 0, :], po[:, 0:512])
                    nc.vector.tensor_copy(obj2[:, t, 1, :], po[:, 512:1024])
                    nc.sync.dma_start(out2_t[:, t, :, :], obj2[:, t, :, :])
                    return
                t1 = epi.tile([P, C_IN], F32, tag="t1")
                nc.scalar.activation(t1[:], po[:, 0:512], Copy,
                                     scale=rrec[0][:, qq : qq + 1])
                nc.vector.scalar_tensor_tensor(
                    obj[:, t, :], po[:, 512:1024], rrec[1][:, qq : qq + 1],
                    t1[:], AluOpType.mult, AluOpType.add,
                )
                if t == QTJ - 1:
                    nc.sync.dma_start(
                        out_t[jj * QH : (jj + 1) * QH, :].rearrange(
                            "(t p) m -> p t m", p=P),
                        obj[:],
                    )

            def av_emit(kt, op0, op1, v, ptpair):
                for h in range(2):
                    nc.tensor.matmul(
                        [op0, op1][h][:, :],
                        v[:, kt % 2, (C + 1) * h : (C + 1) * (h + 1)],
                        ptpair[:, kt % 2, QH * h : QH * (h + 1)],
                        start=(kt == 0), stop=(kt == KT - 1),
                    )

            # ---------------- attention ----------------
            av_q = []  # delayed attention-V matmuls (software pipelining)
            post = []  # deferred previous-chunk epilogue work, 1 item/iter
            for j in range(NJ):
                jsl = slice(j * QH, (j + 1) * QH)
                op0 = opp.tile([C + 1, QH], F32, tag="op0", name=f"op0_{j}")
                op1 = opp.tile([C + 1, QH], F32, tag="op1", name=f"op1_{j}")
                for kt in range(KT):
                    ktsl = slice(kt * P, (kt + 1) * P)
                    if kt % 2 == 0:
                        ebt = ebp.tile([P, 2, 2 * QH], BF16, tag="eb")
                        nc.sync.dma_start(ebt[:], eb_t[kt : kt + 2, :, j, :, :]
                                          .rearrange("k p h q -> p k (h q)"))
                        ptpair = ptp.tile([P, 2, 2 * QH], BF16, tag="pt")
                    sp = spp.tile([P, 1024], F32, tag="sp")
                    nc.tensor.matmul(sp[:, 0:QH], kT[0:C, ktsl], qT[0:C, jsl],
                                     start=True, stop=True)
                    nc.tensor.matmul(sp[:, QH : 2 * QH], kT[C:P, ktsl], qT[C:P, jsl],
                                     start=True, stop=True)
                    ex = pexp.tile([P, 2 * QH], BF16, tag="ex")
                    nc.scalar.activation(ex[:], sp[:], Exp)
                    nc.vector.tensor_mul(ptpair[:, kt % 2, :], ebt[:, kt % 2, :], ex[:])
                    av_q.append((kt, op0, op1, vm[kt // 2], ptpair))
                    if len(av_q) > 2:
                        av_emit(*av_q.pop(0))
                    if j == 0 and kt == 1:
                        nc.sync.dma_start(xt_sb[:, :, QH:S], x_t[:, :, QH:S])
                    if fillers:
                        fillers.pop(0)()
                    elif (j, kt) in sched:
                        sched.pop((j, kt))()
                    else:
                        # dependency-free weight loads keep PE duty high once
                        # projection fillers run dry (HAM gate holds k=8)
                        for _ in range(3):
                            nc.tensor.ldweights(wall_sb[:, 0, 0, :])
                    if post and kt >= 1:
                        post.pop(0)()
                for key in [k for k in sorted(sched) if k[0] == j]:
                    sched.pop(key)()  # un-hit slots (short KT): emit now
                while av_q:
                    av_emit(*av_q.pop(0))
                # dependency-free weight loads keep the PE stream occupied
                # through the chunk-boundary bubble so the HAM clock gate
                # does not drop to half speed.
                for _ in range(10):
                    nc.tensor.ldweights(wall_sb[:, 0, 0, :])

                # epilogue: rowsum -> per-q-tile columns via a DRAM round trip
                # (a single-row reciprocal would run on one DVE lane),
                # reciprocal on 128 lanes. The gate multiply and the output
                # projection are deferred into the next chunk's loop so the
                # DVE burst never starves the PE at the chunk boundary.
                for h, op_ in enumerate((op0, op1)):
                    rs = epi.tile([1, QH], F32, tag="rs")
                    nc.scalar.copy(rs[:], op_[C : C + 1, :])
                    if j == NJ - 1:
                        nc.sync.dma_start(rs2_t[h : h + 1, :], rs[:])
                        continue
                    dscr = drp.tile([1, QH], F32, tag="dscr")
                    nc.sync.dma_start(dscr[:], rs[:])
                    nc.sync.dma_start(
                        rrec[h][:, j * QTJ : (j + 1) * QTJ],
                        dscr[0, :].rearrange("(t p) -> p t", p=P),
                    )
                    nc.vector.reciprocal(rrec[h][:, j * QTJ : (j + 1) * QTJ],
                                         rrec[h][:, j * QTJ : (j + 1) * QTJ])
                obj = outs.tile([P, QTJ, C_IN], BF16, tag="obj")

                def goun(h, op_, jj):
                    jjsl = slice(jj * QH, (jj + 1) * QH)
                    return lambda: nc.vector.tensor_mul(
                        goun2[C * h : C * (h + 1), jjsl], op_[0:C, :],
                        gT[h][:, jjsl])

                post = [goun(0, op0, j), goun(1, op1, j)]
                post += [lambda jj=j, t=t, o=obj: out_qtile(jj, t, o)
                         for t in range(QTJ)]
            while post:
                post.pop(0)()
                # keep the PE stream occupied through the drain so the final
                # projection matmuls run at full clock
                for _ in range(4):
                    nc.tensor.ldweights(wall_sb[:, 0, 0, :])

    return nc


_NC_CACHE = {}


def _get_nc(KT):
    if KT not in _NC_CACHE:
        _NC_CACHE[KT] = _build_nc(KT)
    return _NC_CACHE[KT]


def _prepare_core(c, KT, x, bias, attention_mask, Wq, Wk, Wv, Wg, bg, Wo):
    KPP = KT * P
    b = c // 4
    h1 = 2 * (c % 4)
    h2 = h1 + 1
    sl1 = slice(h1 * C, (h1 + 1) * C)
    sl2 = slice(h2 * C, (h2 + 1) * C)

    idx = np.where(attention_mask[b] > 0)[0]
    nk = idx.size

    # x^T in partition-major layout [P, NCI, S]
    xt = np.ascontiguousarray(
        x[b].T.reshape(NCI, P, S).transpose(1, 0, 2)).astype(NBF)
    xkp = np.zeros((KPP, C_IN), dtype=np.float32)
    xkp[:nk] = x[b][idx]
    xk = np.ascontiguousarray(
        xkp.T.reshape(NCI, P, KPP).transpose(1, 0, 2)).astype(NBF)

    def wsel(W, scale=1.0):
        w = np.concatenate([W[:, sl1], W[:, sl2]], axis=1)
        if scale != 1.0:
            w = w * scale
        return w.reshape(NCI, P, P).transpose(1, 0, 2)  # [P, NCI, P]

    wall = np.ascontiguousarray(np.stack(
        [wsel(Wq, 1.0 / np.sqrt(C)), wsel(Wk), wsel(Wg), wsel(Wv)],
        axis=1)).astype(NBF)  # [P, 4, NCI, P]
    csts = np.empty((P, 2), dtype=np.float32)
    csts[:, 0] = 0.5 * np.concatenate([bg[sl1], bg[sl2]])
    csts[:, 1] = 1.0

    # exp of the transposed pair bias, compacted to kept keys; padding rows
    # stay 0 so they drop out of numerator and denominator alike.
    e = np.zeros((KPP, NJ, 2, QH), dtype=NBF)
    for hi, hh in enumerate((h1, h2)):
        bt = np.exp(bias[b, hh].T[idx])  # [nk, S] f32
        e[:nk, :, hi, :] = bt.reshape(nk, NJ, QH).astype(NBF)
    eb = np.ascontiguousarray(e.reshape(KT, P, NJ, 2, QH))

    wo = np.concatenate([Wo[sl1, :], Wo[sl2, :]], 0).astype(NBF)

    return {
        "xt": xt,
        "xk": xk,
        "wall": wall,
        "csts": csts,
        "ebias": eb,
        "wo": wo,
    }


def _run(inputs, trace=False, **kw):
    x = np.asarray(inputs["x"], dtype=np.float32)
    bias = np.asarray(inputs["bias"], dtype=np.float32)
    attention_mask = np.asarray(inputs["attention_mask"])
    Wq = np.asarray(inputs["Wq"], dtype=np.float32)
    Wk = np.asarray(inputs["Wk"], dtype=np.float32)
    Wv = np.asarray(inputs["Wv"], dtype=np.float32)
    Wg = np.asarray(inputs["Wg"], dtype=np.float32)
    bg = np.asarray(inputs["bg"], dtype=np.float32)
    Wo = np.asarray(inputs["Wo"], dtype=np.float32)
    bo = np.asarray(inputs["bo"], dtype=np.float32)

    nk_max = int(max((attention_mask[b] > 0).sum() for b in range(B)))
    KT = max(2, 2 * math.ceil(nk_max / (2 * P)))  # even tile count

    in_maps = [None] * 8

    def prep(c):
        in_maps[c] = _prepare_core(c, KT, x, bias, attention_mask,
                                   Wq, Wk, Wv, Wg, bg, Wo)

    threads = [threading.Thread(target=prep, args=(c,)) for c in range(8)]
    for t in threads:
        t.start()
    for t in threads:
        t.join()

    nc = _get_nc(KT)
    res = run_bass_kernel_spmd(nc, in_maps, core_ids=list(range(8)), trace=trace, **kw)

    out = np.empty((B, S, C_IN), dtype=np.float32)
    for b in range(B):
        acc = res.results[4 * b]["out"].astype(np.float32)
        for c in range(4 * b + 1, 4 * b + 4):
            acc = acc + res.results[c]["out"]
        # last q-chunk arrives as unnormalized per-head Wo partials
        tail = np.zeros((QTJ, P, C_IN), dtype=np.float32)
        for c in range(4 * b, 4 * b + 4):
            o2 = res.results[c]["out2"].astype(np.float32)  # [P,QTJ,2,CIN]
            rr = 1.0 / res.results[c]["rsum2"].reshape(2, QTJ, P)  # [2,t,p]
            tail += (o2[:, :, 0, :] * rr[0].T[:, :, None]
                     + o2[:, :, 1, :] * rr[1].T[:, :, None]).transpose(1, 0, 2)
        acc[(NJ - 1) * QH : S] = tail.reshape(QH, C_IN)
        out[b] = acc + bo[None, :]
    return out, res


def kernel(**inputs) -> np.ndarray:
    return _run(inputs)[0]
